# revision 1
# baseline (speedup 1.0000x reference)
"""DCT Frequency Splitter — Trainium2 Bass kernel.

Math: FFT2 -> mask -> IFFT2 -> real is a linear operator on the 196 patch
tokens (per channel).  low_sp = A @ patches with A = Re(Finv diag(m) F)
(196x196, real, built on host from the 4 mask params).  Since
high_mask = 1 - mask(high_params):  high_sp = patches - C @ patches with
C = A when low/high params coincide (the common case; then one matmul
feeds both outputs).  The token-mean for the gate MLP is obtained for free
by stacking a ones/196 row onto A, so the whole FFT pipeline plus gate is
a single [197,196] x [196,768] matmul per image plus a tiny batched MLP.

Sharding: pure data parallel, batch 128 -> 16 per core across 8 cores.
"""

import os
import numpy as np

import concourse.bass as bass
import concourse.bacc as bacc_mod
import concourse.mybir as mybir
import concourse.tile as tile
from concourse.bass_utils import run_bass_kernel_spmd
from concourse.tile_rust import add_dep_helper

H, W = 14, 14
B, N, D = 128, 197, 768
P = 196  # patch tokens
NCORES = 8
BS = B // NCORES  # batches per core

# tunables (env overridable for experiments)
GRP = int(os.environ.get("KRN_GROUP", "3"))       # gate MLP group size
MM_DT = os.environ.get("KRN_MM_DT", "f32")         # f32 | f32r
BUFX = int(os.environ.get("KRN_BUFX", "12"))
BUFO = int(os.environ.get("KRN_BUFO", "10"))
DBG_NOGATE = bool(int(os.environ.get("KRN_NOGATE", "0")))
DBG_DMAONLY = bool(int(os.environ.get("KRN_DMAONLY", "0")))
F32 = mybir.dt.float32


def _freq_mask_np(params, low):
    ch, cw, radius, sharp = [np.float64(v) for v in np.asarray(params)]
    y = np.arange(H, dtype=np.float64)
    x = np.arange(W, dtype=np.float64)
    d2 = (y[:, None] - ch) ** 2 + (x[None, :] - cw) ** 2
    dist = np.sqrt(d2 + 1e-12)
    s = np.clip(sharp, 0.5, 10.0)
    r = np.clip(radius, 1.0, min(H, W) / 2.0)
    m = np.exp(-((dist / r) ** s))
    return m if low else 1.0 - m


def _conv_operator(mask):
    """Real 196x196 operator equivalent to ifft2(fft2(img)*mask).real."""
    F_H = np.exp(-2j * np.pi * np.outer(np.arange(H), np.arange(H)) / H)
    F_W = np.exp(-2j * np.pi * np.outer(np.arange(W), np.arange(W)) / W)
    Fi_H = np.conj(F_H) / H
    Fi_W = np.conj(F_W) / W
    op = np.kron(Fi_H, Fi_W) @ np.diag(mask.ravel()) @ np.kron(F_H, F_W)
    return np.real(op)


def _mm_ap(ap):
    if MM_DT == "f32r":
        return ap.bitcast(mybir.dt.float32r)
    return ap


def _build_program(consts, share_Y, b2lo, b2hi, alo, ahi):
    nc = bacc_mod.Bacc(None)

    xs_h = nc.dram_tensor("xs", [BS, N, D], F32, kind="ExternalInput")
    lo_h = nc.dram_tensor("lo", [BS, N, D], F32, kind="ExternalOutput")
    hi_h = nc.dram_tensor("hi", [BS, N, D], F32, kind="ExternalOutput")

    ch = {k: nc.inline_tensor(v, name=f"c_{k}") for k, v in consts.items()}

    Copy = mybir.ActivationFunctionType.Copy
    Relu = mybir.ActivationFunctionType.Relu
    Sig = mybir.ActivationFunctionType.Sigmoid

    with tile.TileContext(nc) as tc:
        with (
            tc.tile_pool(name="consts", bufs=1) as cp,
            tc.tile_pool(name="xp", bufs=BUFX) as xp,
            tc.tile_pool(name="outp", bufs=BUFO) as outp,
            tc.tile_pool(name="gp", bufs=2) as gp,
            tc.tile_pool(name="pm", bufs=(3 if share_Y else 2), space="PSUM") as pm,
            tc.tile_pool(name="pmz", bufs=1, space="PSUM") as pmz,
            tc.tile_pool(name="par", bufs=(2 if share_Y else 1), space="PSUM") as par,
        ):
            # ---- load constants to SBUF
            def cload(key):
                arr = consts[key]
                t = cp.tile(list(arr.shape), F32, tag=key)
                nc.sync.dma_start(out=t[:], in_=ch[key][...])
                return t

            # matmul-critical weights in ONE blob DMA so the first batch's
            # matmuls queue behind a single descriptor slot; everything the
            # gate MLP needs is a second blob deferred until after the first
            # group's data loads (first use is one group later)
            wb = cload("wtblob")        # [128, 410]: wt_lo | wt_hi | ident
            wt_lo = wb[:, 0:197]        # M'^T rows 0:128 (tokens 0..127)
            wt_hi = wb[0:69, 197:394]   # M'^T rows 128:197
            ident = wb[0:16, 394:410]
            if not share_Y:
                ct_lo = cload("ct_lo")  # [128, 197]
                ct_hi = cload("ct_hi")  # [69, 197]
            gate_consts = {}

            def load_deferred():
                gb = cload("gblob")     # [128, 1620] packed gate constants
                gate_consts["w1c"] = gb[:, 0:1152].rearrange(
                    "p (a b) -> p a b", a=6)             # [128, 6, 192]
                gate_consts["b1c"] = gb[0:1, 1152:1344]  # [1, 192]
                gate_consts["w2c0"] = gb[:, 1344:1346]   # [128, 2]
                gate_consts["w2c1"] = gb[0:64, 1346:1348]  # [64, 2]
                gate_consts["ones1"] = gb[0:1, 1348:1364]  # [1, 16]
                gate_consts["alr"] = gb[0:1, 1364:1492]  # [1,128] sig(alpha_low)
                gate_consts["ahr"] = gb[0:1, 1492:1620]  # [1,128] sig(alpha_high)
                # CLS passthrough for all batches in two strided DMAs
                nc.sync.dma_start(out=lo_h[:, 0:1, :], in_=xs_h[:, 0:1, :])
                nc.sync.dma_start(out=hi_h[:, 0:1, :], in_=xs_h[:, 0:1, :])

            n_groups = (BS + GRP - 1) // GRP
            NSPLIT = [(0, 512), (512, 768)]

            def gate_and_store(bs, Gn, arena, per_b):
                """Gate MLP + scales + stores for a finished group, traced
                one group late so the chain hides behind the next group's
                matmul stream."""
                # token means (lo_a row 0) -> transposed gT columns
                for j, b in enumerate(bs):
                    lo_a = per_b[b][2]
                    for c in range(6):
                        nc.tensor.transpose(
                            arena[:, c * 16 + j:c * 16 + j + 1],
                            lo_a[0:1, c * 128:(c + 1) * 128],
                            ident[0:1, 0:1])
                gT = gp.tile([128, 6, 16], F32, tag="gT")
                nc.vector.tensor_copy(
                    gT[:].rearrange("p a b -> p (a b)"), arena[:, 0:96])

                h_ps = arena[0:16, 96:288]
                for c in range(6):
                    nc.tensor.matmul(h_ps[0:Gn, :], _mm_ap(gT[:, c, 0:Gn]),
                                     _mm_ap(gate_consts["w1c"][:, c, :]), start=(c == 0),
                                     stop=False)
                nc.tensor.matmul(h_ps[0:Gn, :], _mm_ap(gate_consts["ones1"][0:1, 0:Gn]),
                                 _mm_ap(gate_consts["b1c"][0:1, :]), start=False, stop=True)
                hs = gp.tile([16, 192], F32, tag="hs")
                nc.vector.tensor_relu(hs[0:Gn, :], h_ps[0:Gn, :])

                hT = gp.tile([128, 2, 16], F32, tag="hT")
                nc.tensor.transpose(arena[:, 288:288 + Gn], hs[0:Gn, 0:128],
                                    ident[0:Gn, 0:Gn])
                nc.tensor.transpose(arena[0:64, 304:304 + Gn],
                                    hs[0:Gn, 128:192], ident[0:Gn, 0:Gn])
                nc.vector.tensor_copy(hT[:].rearrange("p a b -> p (a b)"),
                                      arena[:, 288:320])

                # final layer: two M=1 matmuls (gate rows at partition 0);
                # b2 folds into the sigmoid bias, alpha into a post-scale;
                # rows then replicated across partitions via K=1 matmuls
                crows = []
                for col, b2f, af in ((0, b2lo, alo), (1, b2hi, ahi)):
                    g_ps = arena[0:1, 320 + 16 * col:336 + 16 * col]
                    nc.tensor.matmul(g_ps[:, 0:Gn], _mm_ap(gate_consts["w2c0"][:, col:col + 1]),
                                     _mm_ap(hT[:, 0, 0:Gn]), start=True,
                                     stop=False)
                    nc.tensor.matmul(g_ps[:, 0:Gn], _mm_ap(gate_consts["w2c1"][:, col:col + 1]),
                                     _mm_ap(hT[0:64, 1, 0:Gn]), start=False,
                                     stop=True)
                    cr = gp.tile([1, 16], F32, tag=f"crow{col}")
                    nc.scalar.activation(cr[:, 0:Gn], g_ps[:, 0:Gn], Sig,
                                         bias=b2f)
                    crows.append(cr)
                # replication matmuls against alpha-scaled ones rows fold the
                # alpha multiply in; one copy lands both gate vectors
                for col, wrow in ((0, "alr"), (1, "ahr")):
                    nc.tensor.matmul(
                        arena[:, 352 + 16 * col:352 + 16 * col + Gn],
                        _mm_ap(gate_consts[wrow][0:1, :]),
                        _mm_ap(crows[col][0:1, 0:Gn]),
                        start=True, stop=True)
                crlh = gp.tile([128, 32], F32, tag="crlh")
                nc.vector.tensor_copy(crlh[:], arena[:, 352:384])
                crl = crlh[:, 0:16]
                crh = crlh[:, 16:32]

                # scale in place and store (hi lives in the x tiles)
                for j, b in enumerate(bs):
                    xa, xb, lo_a, lo_b = per_b[b]
                    nc.scalar.activation(lo_a[:], lo_a[:], Copy,
                                         scale=crl[:, j:j + 1])
                    nc.scalar.activation(lo_b[:], lo_b[:], Copy,
                                         scale=crl[0:69, j:j + 1])
                    nc.vector.tensor_scalar_mul(xa[:], xa[:],
                                                crh[:, j:j + 1])
                    nc.vector.tensor_scalar_mul(xb[:], xb[:], crh[0:69, j:j + 1])
                    nc.sync.dma_start(out=lo_h[b, 1:128, :], in_=lo_a[1:128])
                    nc.sync.dma_start(out=lo_h[b, 128:197, :], in_=lo_b[:])
                    nc.sync.dma_start(out=hi_h[b, 1:128, :], in_=xa[1:128])
                    nc.sync.dma_start(out=hi_h[b, 128:197, :], in_=xb[:])

            pending = None   # (bs, Gn, arena, per_b) of previous group
            for g in range(n_groups):
                bs = list(range(g * GRP, min((g + 1) * GRP, BS)))
                Gn = len(bs)
                # per-group psum arena for the gate pipeline (fresh column
                # ranges): 0:96 gT | 96:288 h | 288:320 hT | 320:352 gate |
                # 352:384 replication
                arena = par.tile([128, 512], F32, tag="arena")
                per_b = {}

                # issue the whole group's loads first so they sit AHEAD of
                # the previous group's store burst in the DMA queue FIFOs
                xt = {}
                for b in bs:
                    xa = xp.tile([128, D], F32, tag="xa")
                    xb = xp.tile([69, D], F32, tag="xb")
                    nc.sync.dma_start(out=xa[:], in_=xs_h[b, 0:128, :])
                    nc.sync.dma_start(out=xb[:], in_=xs_h[b, 128:197, :])
                    xt[b] = (xa, xb)
                if g == 0:
                    load_deferred()

                for j, b in enumerate(bs):
                    xa, xb = xt[b]

                    # Y = M' @ x[b]; M' row 0 = token-mean row, rows 1..196
                    # = low-pass operator (CLS column is zero)
                    ylo = pm.tile([128, D], F32, tag="ym")
                    yhi = pm.tile([128, D], F32, tag="ym")
                    for (n0, n1) in NSPLIT:
                        nc.tensor.matmul(ylo[:, n0:n1], _mm_ap(wt_lo[:, 0:128]),
                                         _mm_ap(xa[:, n0:n1]), start=True, stop=False)
                        nc.tensor.matmul(ylo[:, n0:n1], _mm_ap(wt_hi[:, 0:128]),
                                         _mm_ap(xb[:, n0:n1]), start=False, stop=True)
                    for (n0, n1) in NSPLIT:
                        nc.tensor.matmul(yhi[0:69, n0:n1], _mm_ap(wt_lo[:, 128:197]),
                                         _mm_ap(xa[:, n0:n1]), start=True, stop=False)
                        nc.tensor.matmul(yhi[0:69, n0:n1], _mm_ap(wt_hi[:, 128:197]),
                                         _mm_ap(xb[:, n0:n1]), start=False, stop=True)

                    if share_Y:
                        zlo, zhi = ylo, yhi
                    else:
                        zlo = pmz.tile([128, D], F32, tag="zm")
                        zhi = pmz.tile([128, D], F32, tag="zm")
                        for (n0, n1) in NSPLIT:
                            nc.tensor.matmul(zlo[:, n0:n1], _mm_ap(ct_lo[:, 0:128]),
                                             _mm_ap(xa[:, n0:n1]), start=True, stop=False)
                            nc.tensor.matmul(zlo[:, n0:n1], _mm_ap(ct_hi[:, 0:128]),
                                             _mm_ap(xb[:, n0:n1]), start=False, stop=True)
                        for (n0, n1) in NSPLIT:
                            nc.tensor.matmul(zhi[0:69, n0:n1], _mm_ap(ct_lo[:, 128:197]),
                                             _mm_ap(xa[:, n0:n1]), start=True, stop=False)
                            nc.tensor.matmul(zhi[0:69, n0:n1], _mm_ap(ct_hi[:, 128:197]),
                                             _mm_ap(xb[:, n0:n1]), start=False, stop=True)

                    # PSUM -> SBUF (unscaled); hi = x - Y in place in x
                    # tiles; consumed per N-chunk so psum frees sooner
                    lo_a = outp.tile([128, D], F32, tag="lo_a")
                    lo_b = outp.tile([69, D], F32, tag="lo_b")
                    for (n0, n1) in NSPLIT:
                        nc.scalar.activation(lo_a[:, n0:n1], ylo[:, n0:n1], Copy)
                        nc.vector.tensor_sub(xa[:, n0:n1], xa[:, n0:n1],
                                             zlo[:, n0:n1])
                    for (n0, n1) in NSPLIT:
                        nc.scalar.activation(lo_b[:, n0:n1], yhi[0:69, n0:n1],
                                             Copy)
                        nc.vector.tensor_sub(xb[:, n0:n1], xb[:, n0:n1],
                                             zhi[0:69, n0:n1])

                    per_b[b] = (xa, xb, lo_a, lo_b)

                if share_Y:
                    # lag the gate chain one group to hide its latency
                    if pending is not None:
                        gate_and_store(*pending)
                    pending = (bs, Gn, arena, per_b)
                else:
                    # generic path: flush immediately (simpler dependency
                    # structure; correctness over overlap)
                    gate_and_store(bs, Gn, arena, per_b)

            if pending is not None:
                gate_and_store(*pending)
    if not nc.is_finalized():
        nc.finalize()
    return nc


def kernel(x, low_params, high_params, alpha_low, alpha_high,
           w1, b1, w2, b2, cls_token_idx):
    assert int(cls_token_idx) == 0
    x = np.ascontiguousarray(np.asarray(x, dtype=np.float32))
    assert x.shape == (B, N, D)

    lm = _freq_mask_np(low_params, True)
    A = _conv_operator(lm)                       # low operator [196, 196]
    share_Y = np.allclose(np.asarray(low_params, np.float32),
                          np.asarray(high_params, np.float32))
    Cm = A if share_Y else _conv_operator(_freq_mask_np(high_params, True))

    w1 = np.asarray(w1, np.float32)
    sig = lambda v: 1.0 / (1.0 + np.exp(-np.float64(v)))

    def make_consts(OP):
        # M' [197,197]: row 0 = token-mean row, rows 1..196 = OP; CLS col 0
        Mfull = np.zeros((N, N), np.float64)
        Mfull[0, 1:] = 1.0 / P
        Mfull[1:, 1:] = OP
        WT = np.ascontiguousarray(Mfull.T).astype(np.float32)
        wtblob = np.zeros((128, 410), np.float32)
        wtblob[:, 0:197] = WT[0:128]
        wtblob[0:69, 197:394] = WT[128:197]
        wtblob[0:16, 394:410] = np.eye(16, dtype=np.float32)
        gblob = np.zeros((128, 1620), np.float32)
        gblob[:, 0:1152] = w1.reshape(6, 128, 192).transpose(1, 0, 2).reshape(128, 1152)
        gblob[0, 1152:1344] = np.asarray(b1, np.float32)
        gblob[:, 1344:1346] = np.asarray(w2, np.float32)[0:128]
        gblob[0:64, 1346:1348] = np.asarray(w2, np.float32)[128:192]
        gblob[0, 1348:1364] = 1.0
        gblob[0, 1364:1492] = sig(alpha_low)
        gblob[0, 1492:1620] = sig(alpha_high)
        return {"wtblob": wtblob, "gblob": gblob}

    b2v = np.asarray(b2, np.float64).reshape(2)

    def run_once(consts):
        nc = _build_program(consts, True,
                            b2lo=float(b2v[0]), b2hi=float(b2v[1]),
                            alo=float(sig(alpha_low)), ahi=float(sig(alpha_high)))
        xs = x.reshape(NCORES, BS, N, D)
        in_maps = [{"xs": np.ascontiguousarray(xs[c])} for c in range(NCORES)]
        want_trace = bool(int(os.environ.get("KRN_TRACE", "0")))
        try:
            res = run_bass_kernel_spmd(nc, in_maps, core_ids=list(range(NCORES)),
                                       trace=want_trace)
        except ModuleNotFoundError:
            res = run_bass_kernel_spmd(nc, in_maps, core_ids=list(range(NCORES)))
        lo = np.concatenate([r["lo"] for r in res.results], axis=0)
        hi = np.concatenate([r["hi"] for r in res.results], axis=0)
        if getattr(res, "exec_time_ns", None) is not None:
            print(f"HW exec time: {res.exec_time_ns} ns")
        return lo, hi

    if share_Y:
        return run_once(make_consts(A))
    # generic case (never hit by the reference inputs): two passes of the
    # validated single-operator program — lo from the A pass, hi from the C
    # pass (the gate depends only on x, so it is identical in both)
    lo, _ = run_once(make_consts(A))
    _, hi = run_once(make_consts(Cm))
    return lo, hi



# revision 34
# speedup vs baseline: 1.6192x; 1.6192x over previous
"""DCT Frequency Splitter — Trainium2 Bass kernel.

Math: FFT2 -> mask -> IFFT2 -> real is a linear operator on the 196 patch
tokens (per channel).  low_sp = A @ patches with A = Re(Finv diag(m) F)
(196x196, real, built on host from the 4 mask params).  Since
high_mask = 1 - mask(high_params):  high_sp = patches - C @ patches with
C = A when low/high params coincide (the common case; then one matmul
feeds both outputs, and hi = x - lo_unscaled is a vector sub).  The token
mean feeding the gate MLP comes from tiny matmuls against a 1/196 column
(x block stationary), so the gate depends only on the loads.

Sharding: pure data parallel, batch 128 -> 16 per core across 8 cores.

The kernel is I/O bound (29 MB/core vs ~360 GB/s => ~81 us floor), so the
structure keeps the DMA engines dense (cost-model driven):
- main matmuls run as float32r (1 cycle/row at >=256 free cols vs 4 for
  fp32); the BIR verifier requires the producers of f32r-matmul inputs to
  write f32r-typed APs, so the x/weight load DMAs bitcast both sides.
- loads/stores are merged across image groups (every DMACopy holds the
  single HWDGE descriptor unit ~625ns: 100 DMAs = 62us of serialization),
  loads are issued one group ahead of compute, and stores one group late,
  so a store waiting on its scales never head-of-line-blocks a load.
- the gate MLP runs in bf16 off a small weight blob loaded first; its
  scales are ready before the first PSUM evacuation, which folds the lo
  scale into the evacuating activation (hi scales on DVE after the sub).
- sigmoid act table is preloaded at t=0; group sizes taper at the edges
  to shorten the first-store and last-store dependency tails.
"""

import os
import numpy as np

import concourse.bass as bass
import concourse.bacc as bacc_mod
import concourse.mybir as mybir
import concourse.tile as tile
from concourse.bass_utils import run_bass_kernel_spmd

H, W = 14, 14
B, N, D = 128, 197, 768
P = 196  # patch tokens
NCORES = 8
BS = B // NCORES  # batches per core

# tunables (env overridable for experiments)
GRP = int(os.environ.get("KRN_GROUP", "4"))       # images per DMA/gate group
MM_DT = os.environ.get("KRN_MM_DT", "f32r")        # f32 | f32r
BUFX = int(os.environ.get("KRN_BUFX", "3"))        # x-tile generations in flight
BUFO = int(os.environ.get("KRN_BUFO", "2"))        # out-tile generations
F32 = mybir.dt.float32
F32R = mybir.dt.float32r
BF16 = mybir.dt.bfloat16


def _freq_mask_np(params, low):
    ch, cw, radius, sharp = [np.float64(v) for v in np.asarray(params)]
    y = np.arange(H, dtype=np.float64)
    x = np.arange(W, dtype=np.float64)
    d2 = (y[:, None] - ch) ** 2 + (x[None, :] - cw) ** 2
    dist = np.sqrt(d2 + 1e-12)
    s = np.clip(sharp, 0.5, 10.0)
    r = np.clip(radius, 1.0, min(H, W) / 2.0)
    m = np.exp(-((dist / r) ** s))
    return m if low else 1.0 - m


def _conv_operator(mask):
    """Real 196x196 operator equivalent to ifft2(fft2(img)*mask).real."""
    F_H = np.exp(-2j * np.pi * np.outer(np.arange(H), np.arange(H)) / H)
    F_W = np.exp(-2j * np.pi * np.outer(np.arange(W), np.arange(W)) / W)
    Fi_H = np.conj(F_H) / H
    Fi_W = np.conj(F_W) / W
    op = np.kron(Fi_H, Fi_W) @ np.diag(mask.ravel()) @ np.kron(F_H, F_W)
    return np.real(op)


def _mm_ap(ap):
    if MM_DT == "f32r":
        return ap.bitcast(F32R)
    return ap


def _ld_ap(ap):
    # f32r-typed view for DMA producer/consumer APs feeding f32r matmuls
    if MM_DT == "f32r":
        return ap.bitcast(F32R)
    return ap


def _build_program(consts, share_Y, b2lo, b2hi, alo, ahi):
    nc = bacc_mod.Bacc(None)

    xs_h = nc.dram_tensor("xs", [BS, N, D], F32, kind="ExternalInput")
    lo_h = nc.dram_tensor("lo", [BS, N, D], F32, kind="ExternalOutput")
    hi_h = nc.dram_tensor("hi", [BS, N, D], F32, kind="ExternalOutput")

    ch = {k: nc.inline_tensor(v, name=f"c_{k}") for k, v in consts.items()}

    Copy = mybir.ActivationFunctionType.Copy
    Sig = mybir.ActivationFunctionType.Sigmoid

    NSPLIT = [(0, 512), (512, 768)]

    with tile.TileContext(nc) as tc:
        with (
            tc.tile_pool(name="consts", bufs=1) as cp,
            tc.tile_pool(name="xp", bufs=BUFX) as xp,
            tc.tile_pool(name="outp", bufs=BUFO) as outp,
            tc.tile_pool(name="gp", bufs=2) as gp,
            tc.tile_pool(name="pm", bufs=3, space="PSUM") as pm,
            tc.tile_pool(name="par", bufs=2, space="PSUM") as par,
        ):
            # gate-MLP weights load FIRST as a small bf16 blob (~0.5us):
            # every group's gate chain needs them, and as a big f32 blob
            # behind the first loads they head-of-line-blocked the PE queue
            # (gate matmuls precede the mains) for the first ~16us
            # consts ship in two head blobs, biggest first so the 625ns-
            # per-DMA SP issue cadence stays hidden under the transfers.
            # The bf16 gate weights need their OWN dma: an f32r-typed DMA
            # rounds its payload on hardware (that is what the verifier
            # rule is about), which corrupts bit-packed bf16 data.
            gc = cp.tile([128, 1156], BF16, tag="gcrit")
            nc.sync.dma_start(out=gc[:], in_=ch["gcrit"][...])
            wa = cp.tile([128, 412], F32, tag="wtblob")
            nc.sync.dma_start(out=_ld_ap(wa[:]),
                              in_=_ld_ap(ch["wtblob"][...]))
            wt_lo = wa[:, 0:197]        # M'^T rows 0:128 (tokens 0..127)
            wt_hi = wa[0:69, 197:394]   # M'^T rows 128:197
            ident = wa[0:16, 394:410]
            mc_a = wa[:, 410:411]       # token-mean weights (0 at CLS row)
            mc_b = wa[0:69, 411:412]
            gate_consts = {
                "w1c": gc[:, 0:1152].rearrange("p (a b) -> p a b", a=6),
                "w2c0": gc[:, 1152:1154],     # [128, 2]
                "w2c1": gc[0:64, 1154:1156],  # [64, 2]
            }
            # preload the sigmoid activation table during the first loads
            # (the table load costs ~1.3us and would otherwise land on the
            # first gate's critical path)
            warm = gp.tile([1, 1], F32, tag="warm")
            nc.scalar.activation(warm[:], wa[0:1, 394:395], Sig)

            def load_deferred():
                # CLS passthrough for all batches in two strided DMAs
                nc.sync.dma_start(out=lo_h[:, 0:1, :], in_=xs_h[:, 0:1, :])
                nc.sync.dma_start(out=hi_h[:, 0:1, :], in_=xs_h[:, 0:1, :])

            # variable group sizes: small first group so the matmul pipe
            # starts early (per-image loads there, too), small last group so
            # the final gate+store tail is short
            gs_env = os.environ.get("KRN_GS", "")
            if gs_env:
                group_sizes = [int(v) for v in gs_env.split(",")]
                assert sum(group_sizes) == BS and max(group_sizes) <= GRP
            elif GRP == 4 and BS == 16:
                group_sizes = [1, 4, 4, 4, 2, 1]
            else:
                group_sizes = [GRP] * (BS // GRP)
                if BS % GRP:
                    group_sizes.append(BS % GRP)

            def gate_mlp(Gn, arena, xa, xb):
                """Gate MLP for a group, computed straight from the x tiles
                (means via tiny matmuls with the x block stationary and a
                1/196 column moving). Depends only on the loads, so it runs
                concurrently with the group's main matmuls and its scales
                are ready by the time the first evacuation needs them."""
                for j in range(Gn):
                    s = j * D
                    for c in range(6):
                        # plain fp32: free size 1 violates the fp32r ISA
                        # restrictions, and 4 cycles/row is free at this size
                        col = arena[:, c * 16 + j:c * 16 + j + 1]
                        nc.tensor.matmul(col, xa[:, s + c * 128:
                                                 s + (c + 1) * 128],
                                         mc_a, start=True, stop=False)
                        nc.tensor.matmul(col, xb[0:69, s + c * 128:
                                                    s + (c + 1) * 128],
                                         mc_b, start=False, stop=True)
                gT = gp.tile([128, 6, 16], BF16, tag="gT")
                nc.vector.tensor_copy(
                    gT[:].rearrange("p a b -> p (a b)"), arena[:, 0:96])

                h_ps = arena[0:16, 96:288]
                has_b1 = "b1c" in gate_consts
                for c in range(6):
                    nc.tensor.matmul(h_ps[0:Gn, :], gT[:, c, 0:Gn],
                                     gate_consts["w1c"][:, c, :],
                                     start=(c == 0),
                                     stop=(not has_b1 and c == 5))
                if "b1c" in gate_consts:
                    nc.tensor.matmul(h_ps[0:Gn, :],
                                     gate_consts["ones1"][0:1, 0:Gn],
                                     gate_consts["b1c"][0:1, :], start=False,
                                     stop=True)
                hs = gp.tile([16, 192], F32, tag="hs")
                nc.vector.tensor_relu(hs[0:Gn, :], h_ps[0:Gn, :])

                hT = gp.tile([128, 2, 16], BF16, tag="hT")
                nc.tensor.transpose(arena[:, 288:288 + Gn], hs[0:Gn, 0:128],
                                    ident[0:Gn, 0:Gn])
                nc.tensor.transpose(arena[0:64, 304:304 + Gn],
                                    hs[0:Gn, 128:192], ident[0:Gn, 0:Gn])
                nc.vector.tensor_copy(hT[:].rearrange("p a b -> p (a b)"),
                                      arena[:, 288:320])

                # final layer: two M=1 matmuls (gate rows at partition 0);
                # b2 folds into the sigmoid bias, alpha into a post-scale;
                # rows then replicated across partitions via K=1 matmuls
                crows = []
                for col, b2f in ((0, b2lo), (1, b2hi)):
                    g_ps = arena[0:1, 320 + 16 * col:336 + 16 * col]
                    nc.tensor.matmul(g_ps[:, 0:Gn],
                                     gate_consts["w2c0"][:, col:col + 1],
                                     hT[:, 0, 0:Gn], start=True, stop=False)
                    nc.tensor.matmul(g_ps[:, 0:Gn],
                                     gate_consts["w2c1"][:, col:col + 1],
                                     hT[0:64, 1, 0:Gn], start=False, stop=True)
                    cr = gp.tile([1, 16], F32, tag=f"crow{col}")
                    nc.scalar.activation(cr[:, 0:Gn], g_ps[:, 0:Gn], Sig,
                                         bias=b2f)
                    crows.append(cr)
                # replication matmuls against alpha-scaled ones rows fold the
                # alpha multiply in; one copy lands both gate vectors
                for col, wrow in ((0, "alr"), (1, "ahr")):
                    nc.tensor.matmul(
                        arena[:, 352 + 16 * col:352 + 16 * col + Gn],
                        gate_consts[wrow][0:1, :],
                        crows[col][0:1, 0:Gn],
                        start=True, stop=True)
                crlh = gp.tile([128, 32], F32, tag="crlh")
                nc.vector.tensor_copy(crlh[:], arena[:, 352:384])
                return crlh[:, 0:16], crlh[:, 16:32]
            def flush_stores(b0, Gn, tiles):
                """Merged stores, traced TWO groups late (right after the
                next group's loads) so their semaphore waits never head-of-
                line-block load prefetch in the SP DMA queue."""
                lo_ga, lo_gb, hi_ga, hi_gb = tiles
                w = Gn * D
                nc.sync.dma_start(
                    out=lo_h[b0:b0 + Gn, 1:128, :].rearrange("g p d -> p g d"),
                    in_=lo_ga[1:128, 0:w].rearrange("p (g d) -> p g d", g=Gn))
                nc.sync.dma_start(
                    out=lo_h[b0:b0 + Gn, 128:197, :].rearrange("g p d -> p g d"),
                    in_=lo_gb[0:69, 0:w].rearrange("p (g d) -> p g d", g=Gn))
                nc.sync.dma_start(
                    out=hi_h[b0:b0 + Gn, 1:128, :].rearrange("g p d -> p g d"),
                    in_=hi_ga[1:128, 0:w].rearrange("p (g d) -> p g d", g=Gn))
                nc.sync.dma_start(
                    out=hi_h[b0:b0 + Gn, 128:197, :].rearrange("g p d -> p g d"),
                    in_=hi_gb[0:69, 0:w].rearrange("p (g d) -> p g d", g=Gn))

            def trace_loads(b0, Gn, per_image):
                """Merged group loads (2 DMAs); the first group loads per
                image instead so its matmuls can start as soon as one image
                has arrived."""
                w = Gn * D
                xa = xp.tile([128, GRP * D], F32, tag="xa")
                xb = xp.tile([69, GRP * D], F32, tag="xb")
                if per_image:
                    for j in range(Gn):
                        s = j * D
                        nc.sync.dma_start(
                            out=_ld_ap(xa[:, s:s + D]),
                            in_=_ld_ap(xs_h[b0 + j, 0:128, :]))
                        nc.sync.dma_start(
                            out=_ld_ap(xb[0:69, s:s + D]),
                            in_=_ld_ap(xs_h[b0 + j, 128:197, :]))
                else:
                    nc.sync.dma_start(
                        out=_ld_ap(xa[:, 0:w].rearrange("p (g d) -> p g d",
                                                        g=Gn)),
                        in_=_ld_ap(xs_h[b0:b0 + Gn, 0:128, :].rearrange(
                            "g p d -> p g d")))
                    nc.sync.dma_start(
                        out=_ld_ap(xb[0:69, 0:w].rearrange("p (g d) -> p g d",
                                                           g=Gn)),
                        in_=_ld_ap(xs_h[b0:b0 + Gn, 128:197, :].rearrange(
                            "g p d -> p g d")))
                return xa, xb

            starts = [0]
            for Gn in group_sizes:
                starts.append(starts[-1] + Gn)

            pending_store = None   # group awaiting its output stores
            # loads run one group ahead of compute so they sit in the SP
            # queue ahead of older groups' store bursts: the DMA engines
            # always have eligible load work while a store waits on scales
            xts = {0: trace_loads(starts[0], group_sizes[0], True)}
            if "grow" in ch:
                # generic path (b1 != 0): single-row gate constants (bias
                # row, ones row, alpha rows as bit-packed fp32), slotted
                # between L0 and L1 where the issue cadence has a free slot
                gr = cp.tile([1, 720], BF16, tag="grow")
                nc.sync.dma_start(out=gr[:], in_=ch["grow"][...])
                galr = gr[0:1, 208:720].bitcast(F32)   # [1, 256] fp32 view
                gate_consts["b1c"] = gr[0:1, 0:192]
                gate_consts["ones1"] = gr[0:1, 192:208]
                gate_consts["alr"] = galr[:, 0:128]
                gate_consts["ahr"] = galr[:, 128:256]
            else:
                # b1 == 0 (the reference): no bias matmul, and the alpha
                # replication rows are pure constants -> memset instead of a
                # DMA, freeing a head HWDGE cadence slot
                alr_t = cp.tile([1, 128], F32, tag="alr")
                nc.vector.memset(alr_t[:], alo)
                ahr_t = cp.tile([1, 128], F32, tag="ahr")
                nc.vector.memset(ahr_t[:], ahi)
                gate_consts["alr"] = alr_t[0:1, :]
                gate_consts["ahr"] = ahr_t[0:1, :]
            for g, Gn in enumerate(group_sizes):
                b0 = starts[g]
                w = Gn * D
                # per-group psum arena for the gate pipeline: 0:96 gT |
                # 96:288 h | 288:320 hT | 320:352 gate | 352:384 replication
                arena = par.tile([128, 512], F32, tag="arena")

                if g + 1 < len(group_sizes):
                    xts[g + 1] = trace_loads(starts[g + 1],
                                             group_sizes[g + 1], False)
                if g == 0:
                    load_deferred()
                if pending_store is not None:
                    flush_stores(*pending_store)
                    pending_store = None
                xa, xb = xts.pop(g)

                lo_ga = outp.tile([128, GRP * D], F32, tag="lo_ga")
                lo_gb = outp.tile([69, GRP * D], F32, tag="lo_gb")
                hi_ga = outp.tile([128, GRP * D], F32, tag="hi_ga")
                hi_gb = outp.tile([69, GRP * D], F32, tag="hi_gb")

                # gate first: independent of the mains, so its latency hides
                # behind them and the scales below never wait
                crl, crh = gate_mlp(Gn, arena, xa, xb)

                for j in range(Gn):
                    s = j * D
                    # Y = M' @ x[b]; M' row 0 = token-mean row, rows 1..196
                    # = low-pass operator (CLS column is zero)
                    ylo = pm.tile([128, D], F32, tag="ym")
                    yhi = pm.tile([69, D], F32, tag="ym")
                    for (n0, n1) in NSPLIT:
                        nc.tensor.matmul(ylo[:, n0:n1],
                                         _mm_ap(wt_lo[:, 0:128]),
                                         _mm_ap(xa[:, s + n0:s + n1]),
                                         start=True, stop=False)
                        nc.tensor.matmul(ylo[:, n0:n1],
                                         _mm_ap(wt_hi[:, 0:128]),
                                         _mm_ap(xb[0:69, s + n0:s + n1]),
                                         start=False, stop=True)
                    for (n0, n1) in NSPLIT:
                        nc.tensor.matmul(yhi[0:69, n0:n1],
                                         _mm_ap(wt_lo[:, 128:197]),
                                         _mm_ap(xa[:, s + n0:s + n1]),
                                         start=True, stop=False)
                        nc.tensor.matmul(yhi[0:69, n0:n1],
                                         _mm_ap(wt_hi[:, 128:197]),
                                         _mm_ap(xb[0:69, s + n0:s + n1]),
                                         start=False, stop=True)

                    # PSUM -> SBUF with the lo gate scale folded into the
                    # evacuation; hi = (x - Y) then scaled in place
                    nc.scalar.activation(lo_ga[:, s:s + D], ylo[:], Copy,
                                         scale=crl[:, j:j + 1])
                    nc.vector.tensor_sub(hi_ga[:, s:s + D], xa[:, s:s + D],
                                         ylo[:])
                    nc.vector.tensor_scalar_mul(hi_ga[:, s:s + D],
                                                hi_ga[:, s:s + D],
                                                crh[:, j:j + 1])
                    nc.scalar.activation(lo_gb[0:69, s:s + D], yhi[0:69, :],
                                         Copy, scale=crl[0:69, j:j + 1])
                    nc.vector.tensor_sub(hi_gb[0:69, s:s + D],
                                         xb[0:69, s:s + D], yhi[0:69, :])
                    nc.vector.tensor_scalar_mul(hi_gb[0:69, s:s + D],
                                                hi_gb[0:69, s:s + D],
                                                crh[0:69, j:j + 1])

                pending_store = (b0, Gn, (lo_ga, lo_gb, hi_ga, hi_gb))

            if pending_store is not None:
                flush_stores(*pending_store)
    if not nc.is_finalized():
        nc.finalize()
    return nc


def kernel(x, low_params, high_params, alpha_low, alpha_high,
           w1, b1, w2, b2, cls_token_idx):
    assert int(cls_token_idx) == 0
    x = np.ascontiguousarray(np.asarray(x, dtype=np.float32))
    assert x.shape == (B, N, D)

    lm = _freq_mask_np(low_params, True)
    A = _conv_operator(lm)                       # low operator [196, 196]
    share_Y = np.allclose(np.asarray(low_params, np.float32),
                          np.asarray(high_params, np.float32))
    Cm = A if share_Y else _conv_operator(_freq_mask_np(high_params, True))

    w1 = np.asarray(w1, np.float32)
    sig = lambda v: 1.0 / (1.0 + np.exp(-np.float64(v)))

    def make_consts(OP):
        # M' [197,197]: row 0 = token-mean row, rows 1..196 = OP; CLS col 0
        Mfull = np.zeros((N, N), np.float64)
        Mfull[0, 1:] = 1.0 / P
        Mfull[1:, 1:] = OP
        WT = np.ascontiguousarray(Mfull.T).astype(np.float32)
        import ml_dtypes
        wtblob = np.zeros((128, 412), np.float32)
        wtblob[:, 0:197] = WT[0:128]
        wtblob[0:69, 197:394] = WT[128:197]
        wtblob[0:16, 394:410] = np.eye(16, dtype=np.float32)
        wtblob[1:128, 410] = 1.0 / P   # token-mean weights (CLS row 0 = 0)
        wtblob[0:69, 411] = 1.0 / P
        gcrit = np.zeros((128, 1156), np.float32)
        gcrit[:, 0:1152] = w1.reshape(6, 128, 192).transpose(1, 0, 2).reshape(128, 1152)
        gcrit[:, 1152:1154] = np.asarray(w2, np.float32)[0:128]
        gcrit[0:64, 1154:1156] = np.asarray(w2, np.float32)[128:192]
        out = {"wtblob": wtblob, "gcrit": gcrit.astype(ml_dtypes.bfloat16)}
        if np.any(np.asarray(b1, np.float32)):
            grow = np.zeros((1, 720), np.float32)
            grow[0, 0:192] = np.asarray(b1, np.float32)
            grow[0, 192:208] = 1.0
            growb = grow.astype(ml_dtypes.bfloat16)
            galr = np.zeros((1, 256), np.float32)
            galr[0, 0:128] = sig(alpha_low)
            galr[0, 128:256] = sig(alpha_high)
            growb[0, 208:720] = galr.view(ml_dtypes.bfloat16)
            out["grow"] = growb
        return out

    b2v = np.asarray(b2, np.float64).reshape(2)

    def run_once(consts):
        nc = _build_program(consts, True,
                            b2lo=float(b2v[0]), b2hi=float(b2v[1]),
                            alo=float(sig(alpha_low)), ahi=float(sig(alpha_high)))
        xs = x.reshape(NCORES, BS, N, D)
        in_maps = [{"xs": np.ascontiguousarray(xs[c])} for c in range(NCORES)]
        want_trace = bool(int(os.environ.get("KRN_TRACE", "0")))
        try:
            res = run_bass_kernel_spmd(nc, in_maps, core_ids=list(range(NCORES)),
                                       trace=want_trace)
        except ModuleNotFoundError:
            res = run_bass_kernel_spmd(nc, in_maps, core_ids=list(range(NCORES)))
        lo = np.concatenate([r["lo"] for r in res.results], axis=0)
        hi = np.concatenate([r["hi"] for r in res.results], axis=0)
        if getattr(res, "exec_time_ns", None) is not None:
            print(f"HW exec time: {res.exec_time_ns} ns")
        return lo, hi

    if share_Y:
        return run_once(make_consts(A))
    # generic case (never hit by the reference inputs): two passes of the
    # validated single-operator program — lo from the A pass, hi from the C
    # pass (the gate depends only on x, so it is identical in both)
    lo, _ = run_once(make_consts(A))
    _, hi = run_once(make_consts(Cm))
    return lo, hi
